# revision 2
# baseline (speedup 1.0000x reference)
"""Multi-head attention (QKV proj + RoPE + causal SDPA + out proj) on 8 TRN2 cores.

V2: all inputs are pre-cast to bf16 and pre-shuffled on the host into
[128, ...] partition-major contiguous layouts, so every load is a single
full-rate DMA straight into its persistent SBUF tile — no staging buffers, no
on-device dtype-conversion copies.

Sharding: core c = 4*b + g handles batch b (of 2) and head-group g (of 4, i.e.
4 heads = 512 feature dims). Host sums the 4 per-group partials per batch
(the "all-reduce" for the row-sharded w_o).

Device dataflow is feature-major: projections produce qT/kT in [head_dim, seq]
layout and v in [seq, head_dim] layout, matching the S^T = K.Q^T and
O^T = V^T.P^T matmuls — no transposes anywhere on device.
"""

import os
import sys

import numpy as np

sys.path.insert(0, "/opt/trn_rl_repo")

EMB = 2048
SEQ = 2048
N_HEAD = 16
HD = 128
BATCH = 2
N_CORES = 8
GROUPS = 4  # head groups (tensor-parallel dimension)
HPG = N_HEAD // GROUPS  # heads per group = 4
DPG = HPG * HD  # feature dims per group = 512
NE = EMB // 128  # 16 e-blocks
SCALE = float(HD) ** -0.5


def _host_tables(seq):
    """cos / sign-folded sin RoPE tables in [d, s] layout + triangle mask."""
    d = HD
    inv = 1.0 / (10000.0 ** (np.arange(0, d, 2, dtype=np.float64) / d))  # [64]
    pos = np.arange(seq, dtype=np.float64)[None, :] * inv[:, None]  # [64, s]
    ang = np.concatenate([pos, pos], axis=0)  # [128, s]
    cos_t = np.cos(ang)
    sin = np.sin(ang)
    # rot is built as a plain partition swap (rot[0:64]=q[64:128], rot[64:128]=q[0:64]);
    # the rotate_half sign lives in the sin table instead.
    sinm = np.concatenate([-sin[:64], sin[64:]], axis=0)
    # triangle mask for the diagonal 128x128 block: keep (ko, qo) iff qo >= ko
    ko = np.arange(128)[:, None]
    qo = np.arange(128)[None, :]
    mask_t = (qo >= ko).astype(np.float64)
    return cos_t, sinm, mask_t


def build(seq=SEQ, has_bias=False, reps=1):
    """Build the per-core Bass program. Returns the compiled Bacc module.

    reps>1 repeats the whole kernel body inside one NEFF (device-time
    measurement: amortizes the per-launch dispatch overhead).
    """
    import concourse.bacc as bacc
    import concourse.tile as tile
    from concourse import mybir

    bf16 = mybir.dt.bfloat16
    f32 = mybir.dt.float32

    assert seq % 512 == 0
    nj = seq // 512  # 512-wide q chunks

    nc = bacc.Bacc("TRN2", target_bir_lowering=False, debug=False,
                   num_devices=N_CORES, name="mha8v3")

    # host-preshuffled bf16 inputs: partition-major, contiguous per partition
    xt_d = nc.dram_tensor("xt", [128, nj, NE, 512], bf16, kind="ExternalInput")
    wq_d = nc.dram_tensor("wq", [128, NE, DPG], bf16, kind="ExternalInput")
    wk_d = nc.dram_tensor("wk", [128, NE, DPG], bf16, kind="ExternalInput")
    wv_d = nc.dram_tensor("wv", [128, NE, DPG], bf16, kind="ExternalInput")
    wo_d = nc.dram_tensor("wo", [128, HPG, EMB], bf16, kind="ExternalInput")
    bo_d = nc.dram_tensor("bo", [1, EMB], bf16, kind="ExternalInput")
    cos_d = nc.dram_tensor("cosT", [128, seq], bf16, kind="ExternalInput")
    sinm_d = nc.dram_tensor("sinM", [128, seq], bf16, kind="ExternalInput")
    mask_d = nc.dram_tensor("maskT", [128, 128], bf16, kind="ExternalInput")
    out_d = nc.dram_tensor("out", [seq, EMB], bf16, kind="ExternalOutput")

    with tile.TileContext(nc) as tc:
        for _ in range(reps):
            _emit(nc, tc, tile, mybir, seq, nj,
                  xt_d, wq_d, wk_d, wv_d, wo_d, bo_d, cos_d, sinm_d, mask_d,
                  out_d, has_bias)
    nc.compile()
    return nc


def _emit(nc, tc, tile, mybir, seq, nj,
          xt_d, wq_d, wk_d, wv_d, wo_d, bo_d, cos_d, sinm_d, mask_d, out_d,
          has_bias):
    from contextlib import ExitStack

    f32 = mybir.dt.float32
    bf16 = mybir.dt.bfloat16
    EXP = mybir.ActivationFunctionType.Exp
    nsb = seq // 128

    ctx = ExitStack()
    with ctx:
        persist = ctx.enter_context(tc.tile_pool(name="persist", bufs=1))

        # right-side: tables + projection weights (freed after last projection)
        ph2 = ExitStack()
        ph2_pool = ph2.enter_context(tc.tile_pool(name="ph2", bufs=1, side="right"))
        xt_pool = ph2.enter_context(tc.tile_pool(name="xt", bufs=2, side="right"))

        # ---- DMA emission order tuned for the critical path: x0 + wq + wk
        # first (first projection chains), then wv / tables / later chunks.
        ones_col = persist.tile([128, 1], bf16, name="ones_col")
        nc.vector.memset(ones_col, 1.0)

        def load_xt(j):
            xt_j = xt_pool.tile([128, NE, 512], bf16, name=f"xt_{j}", tag="xt")
            # 8 x 256KB pieces across the two HWDGE queues
            for q in range(8):
                eng = nc.sync if q % 2 else nc.scalar
                eng.dma_start(xt_j[:, 2 * q:2 * q + 2, :],
                              xt_d[:, j, 2 * q:2 * q + 2, :])
            return xt_j

        xt0 = load_xt(0)

        # persistent activations
        kt = persist.tile([128, HPG, seq], bf16, name="kt")    # [d, h, s]
        yt = persist.tile([128, HPG, seq], bf16, name="yt")
        v_sb = persist.tile([128, nsb, DPG], bf16, name="v_sb")  # [s_in, blk, d]

        # projection weights: direct to SBUF, split into ~512KB DMAs spread
        # over both issuing queues so multiple DMA engines run in parallel
        w_sb = {}
        for nm in ("wq", "wk", "wv"):
            w_sb[nm] = ph2_pool.tile([128, NE, DPG], bf16, name=f"{nm}_sb")

        def load_w(nm, wd, wi):
            for q in range(4):
                eng = nc.sync if (wi + q) % 2 else nc.scalar
                eng.dma_start(w_sb[nm][:, 4 * q:4 * q + 4, :],
                              wd[:, 4 * q:4 * q + 4, :])

        load_w("wq", wq_d, 0)
        load_w("wk", wk_d, 1)

        mask_sb = persist.tile([128, 128], bf16, name="mask_sb")
        nc.sync.dma_start(mask_sb, mask_d[:])
        cos_sb = ph2_pool.tile([128, seq], bf16, name="cos_sb")
        sinm_sb = ph2_pool.tile([128, seq], bf16, name="sinm_sb")
        nc.scalar.dma_start(cos_sb, cos_d[:])
        nc.scalar.dma_start(sinm_sb, sinm_d[:])
        load_w("wv", wv_d, 0)

        qtj_pool = ctx.enter_context(tc.tile_pool(name="qtj", bufs=2))
        rope_pool = ctx.enter_context(tc.tile_pool(name="rope", bufs=2))
        pt_pool = ctx.enter_context(tc.tile_pool(name="pt", bufs=4))
        sm_pool = ctx.enter_context(tc.tile_pool(name="sm", bufs=2))
        out_pool = ctx.enter_context(tc.tile_pool(name="outp", bufs=3))

        ps_ctx = ExitStack()
        ps2 = ps_ctx.enter_context(tc.tile_pool(name="ps2", bufs=2, space="PSUM"))
        ps1_ctx = ExitStack()
        ps1 = ps1_ctx.enter_context(tc.tile_pool(name="ps1", bufs=2, space="PSUM"))

        def rope(dst, h, j, proj_ps):
            """dst slice = rope(proj_ps) using cos/sinm tables (chunk j)."""
            sl = slice(j * 512, (j + 1) * 512)
            qs = rope_pool.tile([128, 512], bf16, name=f"qs_{h}_{j}", tag="qs")
            nc.vector.tensor_copy(qs, proj_ps)  # PSUM f32 -> SBUF bf16
            rot = rope_pool.tile([128, 512], bf16, name=f"rot_{h}_{j}", tag="rot")
            # rotate_half as partition-shifted copies (sign folded into sinM)
            nc.vector.tensor_copy(rot[0:64, :], qs[64:128, :])
            nc.vector.tensor_copy(rot[64:128, :], qs[0:64, :])
            nc.vector.tensor_mul(qs, qs, cos_sb[:, sl])      # in-place
            nc.vector.tensor_mul(rot, rot, sinm_sb[:, sl])   # in-place
            nc.vector.tensor_add(dst, qs, rot)

        wo_pool = None
        wo_sb = bo_sb = ones_row = None

        for j in range(nj):
            xt_j = xt0 if j == 0 else load_xt(j)

            def qk_proj():
                qt_j = qtj_pool.tile([128, HPG, 512], bf16, name=f"qt_{j}",
                                     tag="qtj")
                for h in range(HPG):
                    for nm in ("wq", "wk"):
                        pp = ps1.tile([128, 512], f32, name=f"pp_{nm}_{h}_{j}",
                                      tag="proj")
                        for e in range(NE):
                            nc.tensor.matmul(
                                pp, w_sb[nm][:, e, h * 128:(h + 1) * 128],
                                xt_j[:, e, :], start=(e == 0), stop=(e == NE - 1))
                        if nm == "wq":
                            rope(qt_j[:, h, :], h, j, pp)
                        else:
                            rope(kt[:, h, j * 512:(j + 1) * 512], h, j, pp)
                return qt_j

            def v_proj():
                for sb in range(4):
                    i_blk = j * 4 + sb
                    vp = ps1.tile([128, DPG], f32, name=f"vp_{i_blk}", tag="proj")
                    for e in range(NE):
                        nc.tensor.matmul(
                            vp, xt_j[:, e, sb * 128:(sb + 1) * 128],
                            w_sb["wv"][:, e, :], start=(e == 0), stop=(e == NE - 1))
                    nc.vector.tensor_copy(v_sb[:, i_blk, :], vp)

            qt_j = qk_proj()
            v_proj()

            if j == nj - 1:
                # last round: projections done with xT/w/tables -> free the
                # right side and load the out-projection weights (one DMA);
                # overlaps this round's attention.
                ps1_ctx.close()
                ph2.close()
                wo_pool = ctx.enter_context(tc.tile_pool(name="wop", bufs=1))
                wo_sb = wo_pool.tile([128, HPG, EMB], bf16, name="wo_sb")
                for h in range(HPG):
                    eng = nc.sync if h % 2 else nc.scalar
                    eng.dma_start(wo_sb[:, h, :], wo_d[:, h, :])
                if has_bias:
                    ones_row = wo_pool.tile([1, 128], bf16, name="ones_row")
                    nc.vector.memset(ones_row, 1.0)
                    bo_sb = wo_pool.tile([1, EMB], bf16, name="bo_sb")
                    nc.scalar.dma_start(bo_sb, bo_d[:])

            # --- attention for all heads, q-chunk j ---
            for h in range(HPG):
                nblk = 4 * j + 4
                ot = ps2.tile([128, 512], f32, name=f"ot_{h}_{j}", tag="ot")
                rs = ps2.tile([1, 512], f32, name=f"rs_{h}_{j}", tag="rs", bufs=1)
                for i in range(nblk):
                    m = i - 4 * j  # diagonal index (>=0 on the 4 trailing blocks)
                    qoff = max(m, 0) * 128
                    n = 512 - qoff
                    st_ps = ps2.tile([128, 512], f32, name=f"st_{h}_{j}_{i}",
                                     tag="st", bufs=3)
                    nc.tensor.matmul(
                        st_ps[:, 0:n], kt[:, h, i * 128:(i + 1) * 128],
                        qt_j[:, h, qoff:512], start=True, stop=True)
                    pt = pt_pool.tile([128, 512], bf16, name=f"pt_{h}_{j}_{i}",
                                      tag="pt")
                    nc.scalar.activation(pt[:, 0:n], st_ps[:, 0:n], EXP,
                                         scale=SCALE)
                    if m >= 0:  # triangle mask on the leading 128 q columns
                        nc.vector.tensor_mul(pt[:, 0:128], pt[:, 0:128], mask_sb)
                    nc.tensor.matmul(ot[:, qoff:512],
                                     v_sb[:, i, h * 128:(h + 1) * 128], pt[:, 0:n],
                                     start=(i == 0), stop=(i == nblk - 1))
                    nc.tensor.matmul(rs[:, qoff:512], ones_col, pt[:, 0:n],
                                     start=(i == 0), stop=(i == nblk - 1))
                # normalize: yt = ot / rowsum (broadcast along partitions)
                rsf = sm_pool.tile([1, 512], f32, name=f"rsf_{h}_{j}", tag="rsf")
                nc.vector.tensor_copy(rsf, rs)
                nc.vector.reciprocal_approx_fast(rsf, rsf)
                rb = sm_pool.tile([128, 512], f32, name=f"rb_{h}_{j}", tag="rb")
                nc.gpsimd.partition_broadcast(rb, rsf)
                nc.vector.tensor_mul(yt[:, h, j * 512:(j + 1) * 512], ot, rb)

        # ---- output projection (ps2 still open: ps3 gets the 2 freed banks,
        # letting early row-blocks overlap the tail of round-3 attention).
        # bf16 partials: one [128, 2048] tile + one 512KB DMA per row-block.
        with tc.tile_pool(name="ps3", bufs=2, space="PSUM") as ps3:
            for sb in range(nsb):
                ssl = slice(sb * 128, (sb + 1) * 128)
                ob = out_pool.tile([128, EMB], bf16, name=f"ob_{sb}", tag="ob")
                for ec in range(EMB // 512):
                    esl = slice(ec * 512, (ec + 1) * 512)
                    op = ps3.tile([128, 512], f32, name=f"op_{sb}_{ec}",
                                  tag="op")
                    for h in range(HPG):
                        nc.tensor.matmul(op, yt[:, h, ssl],
                                         wo_sb[:, h, esl],
                                         start=(h == 0),
                                         stop=(not has_bias and h == HPG - 1))
                    if has_bias:
                        nc.tensor.matmul(op, ones_row, bo_sb[:, esl],
                                         start=False, stop=True)
                    # split PSUM->SBUF copies between ACT and DVE
                    if (sb + ec) % 2:
                        nc.scalar.copy(ob[:, esl], op)
                    else:
                        nc.vector.tensor_copy(ob[:, esl], op)
                eng = nc.sync if sb % 2 == 0 else nc.scalar
                eng.dma_start(out_d[ssl, :], ob)
        ps_ctx.close()


_NC_CACHE = {}


def _get_nc(seq=SEQ, has_bias=False):
    key = (seq, has_bias)
    if key not in _NC_CACHE:
        _NC_CACHE[key] = build(seq, has_bias)
    return _NC_CACHE[key]


def make_in_maps(x, w_kv, w_q, w_o, b_o, seq=SEQ):
    """Shard + bf16-cast + partition-shuffle full inputs into 8 per-core dicts."""
    import ml_dtypes

    bf16 = ml_dtypes.bfloat16
    nj = seq // 512
    cos_t, sinm, mask_t = _host_tables(seq)
    cos_t = np.ascontiguousarray(cos_t.astype(bf16))
    sinm = np.ascontiguousarray(sinm.astype(bf16))
    mask_t = np.ascontiguousarray(mask_t.astype(bf16))
    zeros_bo = np.zeros((1, EMB), bf16)

    # xt[p, j, e, s'] = x[b][j*512+s', e*128+p]
    xts = []
    for b in range(BATCH):
        xb = np.asarray(x[b], np.float32).astype(bf16)  # [s, emb]
        t = xb.reshape(nj, 512, NE, 128).transpose(3, 0, 2, 1)
        xts.append(np.ascontiguousarray(t))

    def wshuf(w):  # [emb, DPG] -> [128, NE, DPG]
        return np.ascontiguousarray(
            np.asarray(w, np.float32).astype(bf16).reshape(NE, 128, DPG)
            .transpose(1, 0, 2))

    in_maps = []
    for c in range(N_CORES):
        b, g = divmod(c, GROUPS)
        d0 = g * DPG
        # wo[p, h, e] = w_o[d0 + h*128 + p, e]
        wo_g = np.ascontiguousarray(
            np.asarray(w_o[d0:d0 + DPG, :], np.float32).astype(bf16)
            .reshape(HPG, 128, EMB).transpose(1, 0, 2))
        in_maps.append({
            "xt": xts[b],
            "wq": wshuf(w_q[:, d0:d0 + DPG]),
            "wk": wshuf(w_kv[:, d0:d0 + DPG]),
            "wv": wshuf(w_kv[:, EMB + d0:EMB + d0 + DPG]),
            "wo": wo_g,
            "bo": (np.ascontiguousarray(
                       np.asarray(b_o, np.float32).reshape(1, EMB).astype(bf16))
                   if g == 0 else zeros_bo),
            "cosT": cos_t,
            "sinM": sinm,
            "maskT": mask_t,
        })
    return in_maps


def kernel(x, w_kv, w_q, w_o, b_o):
    from concourse.bass_utils import run_bass_kernel_spmd

    x = np.asarray(x, np.float32)
    nc = _get_nc(SEQ, has_bias=bool(np.any(np.asarray(b_o))))
    in_maps = make_in_maps(x, np.asarray(w_kv, np.float32),
                           np.asarray(w_q, np.float32),
                           np.asarray(w_o, np.float32),
                           np.asarray(b_o, np.float32), SEQ)
    res = run_bass_kernel_spmd(nc, in_maps, core_ids=list(range(N_CORES)))
    parts = [np.asarray(res.results[c]["out"], np.float32)
             for c in range(N_CORES)]
    out = np.stack(
        [parts[0] + parts[1] + parts[2] + parts[3],
         parts[4] + parts[5] + parts[6] + parts[7]], axis=0)
    return out.astype(np.float32)
